# revision 1
# baseline (speedup 1.0000x reference)
"""MoE (top-2 of 8 experts, SwiGLU FFN) Trainium2 kernel.

Strategy (expert-parallel, host-side dispatch):
  - Router (logits -> softmax -> top-2 -> renormalize) runs on host in
    float32 numpy, mirroring the reference math exactly.
  - Tokens are gathered per expert on host, padded to a common capacity C
    (max expert load rounded up to 128), transposed to [D, C], cast bf16.
  - Core e runs the full SwiGLU FFN of expert e over its C tokens:
        yT = Wd^T-contract( silu(WgT x) * (WuT x) )   in [D, C] layout,
    all matmuls in bf16 with fp32 PSUM accumulation.
  - Host scales each expert's output rows by the routing gate and
    scatter-adds into the [B*S, D] result.

Device layouts (per core):
  xt  [1024, C]  bf16   x^T, d-major (contraction dim on partitions)
  wg  [4096, 1024] bf16 f-chunk-tiled: row f*128+p, col d*128+j = WgT[d*128+p, f*128+j]
  wu  [4096, 1024] bf16 same tiling as wg
  wd  [4096, 1024] bf16 W_down^T as-is (row = f, col = d)
  yt  [1024, C]  f32    y^T
"""

import numpy as np
import ml_dtypes
from contextlib import ExitStack

import concourse.bacc as bacc
import concourse.tile as tile
import concourse.mybir as mybir
from concourse.bass_utils import run_bass_kernel_spmd

B, S, D, F, E, TOPK = 4, 2048, 1024, 4096, 8, 2
N_CORES = 8
FC = F // 128  # 32 f-chunks
DC = D // 128  # 8 d-chunks

_cache: dict = {}

# HW-A/B-tuned schedule: down(i) emitted after gate/up(i+1), double-buffered
# hh, wd loads split around block 0's gate/up, deep DMA prefetch.
BEST_KW = dict(
    pipelined=True,
    hh_bufs=2,
    wd_spread=True,
    x_bufs=3,
    w_bufs=8,
    psum_bufs=3,
    op_bufs=2,
    alt_dma=True,  # x/y I/O on SWDGE, weights on HWDGE: -41us queue contention
)


def _route(x_flat: np.ndarray, W_router: np.ndarray):
    """Top-2 routing, float32 numpy mirror of the jax reference."""
    logits = x_flat @ W_router.T  # [T, E] f32
    m = logits.max(-1, keepdims=True)
    p = np.exp((logits - m).astype(np.float32))
    p /= p.sum(-1, keepdims=True)
    idx = np.argsort(-p, axis=-1)[:, :TOPK]  # [T, 2]
    g = np.take_along_axis(p, idx, -1)
    g = (g / g.sum(-1, keepdims=True)).astype(np.float32)
    return idx, g


def _blocks(C: int):
    out = []
    t = 0
    while t < C:
        tb = min(512, C - t)
        out.append((t, tb))
        t += tb
    return out


def _build(
    C: int,
    reps: int = 1,
    hh_bufs: int = 1,
    psum_bufs: int = 2,
    op_bufs: int | None = None,
    x_bufs: int = 2,
    w_bufs: int = 4,
    pipelined: bool = False,
    wd_spread: bool = False,
    alt_dma: bool = False,
):
    """Build + compile the per-core SwiGLU FFN program for capacity C."""
    dt_w = mybir.dt.bfloat16
    f32 = mybir.dt.float32
    nc = bacc.Bacc("TRN2", target_bir_lowering=False, debug=False, num_devices=N_CORES)
    xt = nc.dram_tensor("xt", [D, C], dt_w, kind="ExternalInput")
    wg = nc.dram_tensor("wg", [F, D], dt_w, kind="ExternalInput")
    wu = nc.dram_tensor("wu", [F, D], dt_w, kind="ExternalInput")
    wd = nc.dram_tensor("wd", [F, D], dt_w, kind="ExternalInput")
    yt = nc.dram_tensor("yt", [D, C], f32, kind="ExternalOutput")

    with tile.TileContext(nc) as tc:
        with ExitStack() as ctx:
            wdp = ctx.enter_context(tc.tile_pool(name="wdp", bufs=1))
            wgp = ctx.enter_context(tc.tile_pool(name="wgp", bufs=w_bufs))
            wup = ctx.enter_context(tc.tile_pool(name="wup", bufs=w_bufs))
            xp = ctx.enter_context(tc.tile_pool(name="xp", bufs=x_bufs))
            hp = ctx.enter_context(tc.tile_pool(name="hp", bufs=hh_bufs))
            sp = ctx.enter_context(tc.tile_pool(name="sp", bufs=3))
            yp = ctx.enter_context(tc.tile_pool(name="yp", bufs=3))
            gp = ctx.enter_context(tc.tile_pool(name="gp", bufs=psum_bufs, space="PSUM"))
            up = ctx.enter_context(tc.tile_pool(name="up", bufs=psum_bufs, space="PSUM"))
            op = ctx.enter_context(
                tc.tile_pool(name="op", bufs=op_bufs or psum_bufs, space="PSUM")
            )

            io_eng = nc.gpsimd if alt_dma else nc.sync

            def gate_up_phase(tok0, TB):
                x_sb = xp.tile([128, DC * TB], dt_w, tag="x")
                for d in range(DC):
                    io_eng.dma_start(
                        x_sb[:, d * TB : (d + 1) * TB],
                        xt[d * 128 : (d + 1) * 128, tok0 : tok0 + TB],
                    )
                hh = []
                for f in range(FC):
                    wg_sb = wgp.tile([128, D], dt_w, tag="wgc")
                    nc.sync.dma_start(wg_sb[:], wg[f * 128 : (f + 1) * 128, :])
                    wu_sb = wup.tile([128, D], dt_w, tag="wuc")
                    nc.sync.dma_start(wu_sb[:], wu[f * 128 : (f + 1) * 128, :])
                    g_ps = gp.tile([128, TB], f32, tag="g")
                    u_ps = up.tile([128, TB], f32, tag="u")
                    for d in range(DC):
                        nc.tensor.matmul(
                            g_ps[:],
                            wg_sb[:, d * 128 : (d + 1) * 128],
                            x_sb[:, d * TB : (d + 1) * TB],
                            start=(d == 0),
                            stop=(d == DC - 1),
                        )
                    for d in range(DC):
                        nc.tensor.matmul(
                            u_ps[:],
                            wu_sb[:, d * 128 : (d + 1) * 128],
                            x_sb[:, d * TB : (d + 1) * TB],
                            start=(d == 0),
                            stop=(d == DC - 1),
                        )
                    sg = sp.tile([128, TB], dt_w, tag="sg")
                    nc.scalar.activation(
                        sg[:], g_ps[:], mybir.ActivationFunctionType.Sigmoid
                    )
                    sg2 = sp.tile([128, TB], dt_w, tag="sg2")
                    nc.vector.tensor_mul(sg2[:], sg[:], g_ps[:])
                    h = hp.tile([128, TB], dt_w, tag=f"hh{f}")
                    nc.vector.tensor_mul(h[:], sg2[:], u_ps[:])
                    hh.append(h)
                return hh

            def down_phase(wd_sb, hh, tok0, TB):
                for d in range(DC):
                    y_ps = op.tile([128, TB], f32, tag="y")
                    for f in range(FC):
                        nc.tensor.matmul(
                            y_ps[:],
                            wd_sb[f][:, d * 128 : (d + 1) * 128],
                            hh[f][:],
                            start=(f == 0),
                            stop=(f == FC - 1),
                        )
                    y_sb = yp.tile([128, TB], f32, tag="ysb")
                    nc.vector.tensor_copy(y_sb[:], y_ps[:])
                    io_eng.dma_start(
                        yt[d * 128 : (d + 1) * 128, tok0 : tok0 + TB], y_sb[:]
                    )

            for _rep in range(reps):
                wd_sb = []

                def load_wd(fs):
                    for f in fs:
                        t = wdp.tile([128, D], dt_w, tag=f"wd{f}")
                        nc.sync.dma_start(t[:], wd[f * 128 : (f + 1) * 128, :])
                        wd_sb.append(t)

                if not wd_spread:
                    load_wd(range(FC))

                blocks = _blocks(C)
                if pipelined:
                    # emit g/u of block i+1 before down of block i
                    pend = None  # (hh, tok0, TB)
                    for bi, (tok0, TB) in enumerate(blocks):
                        if wd_spread and bi == 0:
                            load_wd(range(FC // 2))
                        hh = gate_up_phase(tok0, TB)
                        if wd_spread and bi == 0:
                            load_wd(range(FC // 2, FC))
                        if pend is not None:
                            down_phase(wd_sb, *pend)
                        pend = (hh, tok0, TB)
                    down_phase(wd_sb, *pend)
                else:
                    for tok0, TB in blocks:
                        hh = gate_up_phase(tok0, TB)
                        down_phase(wd_sb, hh, tok0, TB)
    nc.compile()
    return nc


def _tile_gate_weights(w_t: np.ndarray) -> np.ndarray:
    """[D, F] -> [F, D] tiled so row f*128+p, col d*128+j = w_t[d*128+p, f*128+j]."""
    return (
        w_t.reshape(DC, 128, FC, 128).transpose(2, 1, 0, 3).reshape(F, D)
    )


def _ffn_host(x_rows, Wg, Wu, Wd):
    """Exact f32 SwiGLU FFN on host for overflow tokens."""
    g = x_rows @ Wg.T
    u = x_rows @ Wu.T
    h = (g / (1.0 + np.exp(-g))) * u
    return h @ Wd.T


# Device capacity policy: prefer a clean multiple of 512 token blocks and
# compute the (tiny) overflow beyond it on host; fall back to padding the
# device capacity up when overflow would be non-negligible.
OVERFLOW_FRAC_MAX = 0.02


def kernel(x, W_router, W_gate, W_up, W_down):
    bf16 = ml_dtypes.bfloat16
    x = np.asarray(x, np.float32)
    W_router = np.asarray(W_router, np.float32)
    W_gate = np.asarray(W_gate, np.float32)
    W_up = np.asarray(W_up, np.float32)
    W_down = np.asarray(W_down, np.float32)

    T = B * S
    x_flat = x.reshape(T, D)
    idx, gates = _route(x_flat, W_router)

    # token lists per expert
    tok_lists = []
    gate_lists = []
    for e in range(E):
        sel = np.nonzero(idx == e)  # (token_rows, k_pos)
        rows = sel[0]
        tok_lists.append(rows)
        gate_lists.append(gates[sel[0], sel[1]])

    max_load = max(len(r) for r in tok_lists)
    C_pad = max(128, int(np.ceil(max_load / 128)) * 128)
    C_512 = max(512, (max_load // 512) * 512)
    overflow = sum(max(0, len(r) - C_512) for r in tok_lists)
    if overflow <= OVERFLOW_FRAC_MAX * T * TOPK:
        C = C_512
    else:
        C = C_pad

    if C not in _cache:
        _cache[C] = _build(C, **BEST_KW)
    nc = _cache[C]

    in_maps = []
    for e in range(E):
        rows = tok_lists[e][:C]
        xg = np.zeros((C, D), np.float32)
        xg[: len(rows)] = x_flat[rows]
        in_maps.append(
            {
                "xt": np.ascontiguousarray(xg.T).astype(bf16),
                "wg": np.ascontiguousarray(
                    _tile_gate_weights(W_gate[e].T.astype(np.float32))
                ).astype(bf16),
                "wu": np.ascontiguousarray(
                    _tile_gate_weights(W_up[e].T.astype(np.float32))
                ).astype(bf16),
                "wd": np.ascontiguousarray(W_down[e].T).astype(bf16),
            }
        )

    try:
        res = run_bass_kernel_spmd(nc, in_maps, core_ids=list(range(N_CORES)))
    except Exception:
        # transient device failures (e.g. NRT exec-unit unrecoverable) have
        # been observed on this tunnel; one retry usually succeeds
        res = run_bass_kernel_spmd(nc, in_maps, core_ids=list(range(N_CORES)))

    out = np.zeros((T, D), np.float32)
    for e in range(E):
        rows = tok_lists[e]
        n_dev = min(len(rows), C)
        y_e = res.results[e]["yt"].T[:n_dev]  # [n_dev, D]
        out[rows[:n_dev]] += gate_lists[e][:n_dev, None] * y_e
        if len(rows) > C:  # overflow tokens -> exact host FFN
            orows = rows[C:]
            y_o = _ffn_host(x_flat[orows], W_gate[e], W_up[e], W_down[e])
            out[orows] += gate_lists[e][C:, None] * y_o
    return out.reshape(B, S, D)



# revision 10
# speedup vs baseline: 1.0341x; 1.0341x over previous
"""MoE (top-2 of 8 experts, SwiGLU FFN) Trainium2 kernel.

Strategy (expert-parallel, host-side dispatch):
  - Router (logits -> softmax -> top-2 -> renormalize) runs on host in
    float32 numpy, mirroring the reference math exactly.
  - Tokens are gathered per expert on host, padded to a common capacity C
    (max expert load rounded up to 128), transposed to [D, C], cast bf16.
  - Core e runs the full SwiGLU FFN of expert e over its C tokens:
        yT = Wd^T-contract( silu(WgT x) * (WuT x) )   in [D, C] layout,
    all matmuls in bf16 with fp32 PSUM accumulation.
  - Host scales each expert's output rows by the routing gate and
    scatter-adds into the [B*S, D] result.

Device layouts (per core):
  xt  [1024, C]  bf16   x^T, d-major (contraction dim on partitions)
  wg  [4096, 1024] bf16 f-chunk-tiled: row f*128+p, col d*128+j = WgT[d*128+p, f*128+j]
  wu  [4096, 1024] bf16 same tiling as wg
  wd  [4096, 1024] bf16 W_down^T as-is (row = f, col = d)
  yt  [1024, C]  f32    y^T
"""

import numpy as np
import ml_dtypes
from contextlib import ExitStack

import concourse.bacc as bacc
import concourse.tile as tile
import concourse.mybir as mybir
from concourse.bass_utils import run_bass_kernel_spmd

B, S, D, F, E, TOPK = 4, 2048, 1024, 4096, 8, 2
N_CORES = 8
FC = F // 128  # 32 f-chunks
DC = D // 128  # 8 d-chunks

_cache: dict = {}

# HW-A/B-tuned schedule: down(i) emitted after gate/up(i+1), double-buffered
# hh, wd loads split around block 0's gate/up, deep DMA prefetch.
BEST_KW = dict(
    pipelined=True,
    hh_bufs=2,
    wd_spread=True,
    x_bufs=3,
    w_bufs=8,
    psum_bufs=3,
    op_bufs=2,
    alt_dma=True,  # x/y I/O on SWDGE, weights on HWDGE: -41us queue contention
    silu_fuse=True,  # ACT Silu + one DVE mul instead of sigmoid + 2 muls: -40us
    big_dma=False,
    gu_interleave=False,
)


def _route(x_flat: np.ndarray, W_router: np.ndarray):
    """Top-2 routing, float32 numpy mirror of the jax reference."""
    logits = x_flat @ W_router.T  # [T, E] f32
    m = logits.max(-1, keepdims=True)
    p = np.exp((logits - m).astype(np.float32))
    p /= p.sum(-1, keepdims=True)
    idx = np.argsort(-p, axis=-1)[:, :TOPK]  # [T, 2]
    g = np.take_along_axis(p, idx, -1)
    g = (g / g.sum(-1, keepdims=True)).astype(np.float32)
    return idx, g


def _blocks(C: int):
    out = []
    t = 0
    while t < C:
        tb = min(512, C - t)
        out.append((t, tb))
        t += tb
    return out


def _build(
    C: int,
    reps: int = 1,
    hh_bufs: int = 1,
    psum_bufs: int = 2,
    op_bufs: int | None = None,
    x_bufs: int = 2,
    w_bufs: int = 4,
    pipelined: bool = False,
    wd_spread: bool = False,
    alt_dma: bool = False,
    silu_fuse: bool = False,
    big_dma: bool = False,
    gu_interleave: bool = False,
    wide: bool = False,
):
    """Build + compile the per-core SwiGLU FFN program for capacity C."""
    if wide and C % 1024 == 0:
        return _build_wide(C, reps=reps, w_bufs=min(w_bufs, 4), op_bufs=op_bufs or 2)
    dt_w = mybir.dt.bfloat16
    f32 = mybir.dt.float32
    nc = bacc.Bacc("TRN2", target_bir_lowering=False, debug=False, num_devices=N_CORES)
    xt = nc.dram_tensor("xt", [D, C], dt_w, kind="ExternalInput")
    wg = nc.dram_tensor("wg", [F, D], dt_w, kind="ExternalInput")
    wu = nc.dram_tensor("wu", [F, D], dt_w, kind="ExternalInput")
    wd = nc.dram_tensor("wd", [F, D], dt_w, kind="ExternalInput")
    yt = nc.dram_tensor("yt", [D, C], f32, kind="ExternalOutput")

    with tile.TileContext(nc) as tc:
        with ExitStack() as ctx:
            wpair_bufs = max(2, w_bufs // 2) if big_dma else w_bufs
            wdp = ctx.enter_context(tc.tile_pool(name="wdp", bufs=1))
            wgp = ctx.enter_context(tc.tile_pool(name="wgp", bufs=wpair_bufs))
            wup = ctx.enter_context(tc.tile_pool(name="wup", bufs=wpair_bufs))
            xp = ctx.enter_context(tc.tile_pool(name="xp", bufs=x_bufs))
            hp = ctx.enter_context(tc.tile_pool(name="hp", bufs=hh_bufs))
            sp = ctx.enter_context(tc.tile_pool(name="sp", bufs=3))
            yp = ctx.enter_context(tc.tile_pool(name="yp", bufs=3))
            gp = ctx.enter_context(tc.tile_pool(name="gp", bufs=psum_bufs, space="PSUM"))
            up = ctx.enter_context(tc.tile_pool(name="up", bufs=psum_bufs, space="PSUM"))
            op = ctx.enter_context(
                tc.tile_pool(name="op", bufs=op_bufs or psum_bufs, space="PSUM")
            )

            io_eng = nc.gpsimd if alt_dma else nc.sync

            xt_r = xt.rearrange("(dc p) t -> p dc t", p=128)
            wg_r = wg.rearrange("(fc p) d -> p fc d", p=128)
            wu_r = wu.rearrange("(fc p) d -> p fc d", p=128)
            wd_r = wd.rearrange("(fc p) d -> p fc d", p=128)

            def gate_up_phase(tok0, TB):
                x_sb = xp.tile([128, DC * TB], dt_w, tag="x")
                if big_dma:
                    io_eng.dma_start(
                        x_sb[:].rearrange("p (dc t) -> p dc t", dc=DC),
                        xt_r[:, :, tok0 : tok0 + TB],
                    )
                else:
                    for d in range(DC):
                        io_eng.dma_start(
                            x_sb[:, d * TB : (d + 1) * TB],
                            xt[d * 128 : (d + 1) * 128, tok0 : tok0 + TB],
                        )
                hh = []
                wg_pair = wu_pair = None
                for f in range(FC):
                    if big_dma:
                        if f % 2 == 0:
                            wg_pair = wgp.tile([128, 2, D], dt_w, tag="wgc")
                            nc.sync.dma_start(wg_pair[:], wg_r[:, f : f + 2, :])
                            wu_pair = wup.tile([128, 2, D], dt_w, tag="wuc")
                            nc.sync.dma_start(wu_pair[:], wu_r[:, f : f + 2, :])
                        k = f % 2
                        gpair, upair = wg_pair, wu_pair
                        wg_sb = lambda d, t=gpair, k=k: t[
                            :, k : k + 1, d * 128 : (d + 1) * 128
                        ]
                        wu_sb = lambda d, t=upair, k=k: t[
                            :, k : k + 1, d * 128 : (d + 1) * 128
                        ]
                    else:
                        wg_t = wgp.tile([128, D], dt_w, tag="wgc")
                        nc.sync.dma_start(wg_t[:], wg[f * 128 : (f + 1) * 128, :])
                        wu_t = wup.tile([128, D], dt_w, tag="wuc")
                        nc.sync.dma_start(wu_t[:], wu[f * 128 : (f + 1) * 128, :])
                        wg_sb = lambda d, t=wg_t: t[:, d * 128 : (d + 1) * 128]
                        wu_sb = lambda d, t=wu_t: t[:, d * 128 : (d + 1) * 128]
                    g_ps = gp.tile([128, TB], f32, tag="g")
                    u_ps = up.tile([128, TB], f32, tag="u")

                    def mm(ps, w_fn, d):
                        nc.tensor.matmul(
                            ps[:],
                            w_fn(d),
                            x_sb[:, d * TB : (d + 1) * TB],
                            start=(d == 0),
                            stop=(d == DC - 1),
                        )

                    if gu_interleave:
                        for d in range(DC):
                            mm(g_ps, wg_sb, d)
                            mm(u_ps, wu_sb, d)
                    else:
                        for d in range(DC):
                            mm(g_ps, wg_sb, d)
                        for d in range(DC):
                            mm(u_ps, wu_sb, d)
                    h = hp.tile([128, TB], dt_w, tag=f"hh{f}")
                    if silu_fuse:
                        sg = sp.tile([128, TB], dt_w, tag="sg")
                        nc.scalar.activation(
                            sg[:], g_ps[:], mybir.ActivationFunctionType.Silu
                        )
                        nc.vector.tensor_mul(h[:], sg[:], u_ps[:])
                    else:
                        sg = sp.tile([128, TB], dt_w, tag="sg")
                        nc.scalar.activation(
                            sg[:], g_ps[:], mybir.ActivationFunctionType.Sigmoid
                        )
                        sg2 = sp.tile([128, TB], dt_w, tag="sg2")
                        nc.vector.tensor_mul(sg2[:], sg[:], g_ps[:])
                        nc.vector.tensor_mul(h[:], sg2[:], u_ps[:])
                    hh.append(h)
                return hh

            def down_phase(wd_sb, hh, tok0, TB):
                for d in range(DC):
                    y_ps = op.tile([128, TB], f32, tag="y")
                    for f in range(FC):
                        if big_dma:
                            w_ap = wd_sb[f // 8][
                                :, (f % 8) : (f % 8) + 1, d * 128 : (d + 1) * 128
                            ]
                        else:
                            w_ap = wd_sb[f][:, d * 128 : (d + 1) * 128]
                        nc.tensor.matmul(
                            y_ps[:],
                            w_ap,
                            hh[f][:],
                            start=(f == 0),
                            stop=(f == FC - 1),
                        )
                    y_sb = yp.tile([128, TB], f32, tag="ysb")
                    nc.vector.tensor_copy(y_sb[:], y_ps[:])
                    io_eng.dma_start(
                        yt[d * 128 : (d + 1) * 128, tok0 : tok0 + TB], y_sb[:]
                    )

            for _rep in range(reps):
                wd_sb = []

                def load_wd(fs):
                    if big_dma:
                        for k in fs:
                            t = wdp.tile(
                                [128, 8, D], dt_w, tag=f"wd{k}", name=f"wd_t{k}"
                            )
                            nc.sync.dma_start(t[:], wd_r[:, 8 * k : 8 * k + 8, :])
                            wd_sb.append(t)
                    else:
                        for f in fs:
                            t = wdp.tile([128, D], dt_w, tag=f"wd{f}", name=f"wd_t{f}")
                            nc.sync.dma_start(t[:], wd[f * 128 : (f + 1) * 128, :])
                            wd_sb.append(t)

                NWD = 4 if big_dma else FC
                if not wd_spread:
                    load_wd(range(NWD))

                blocks = _blocks(C)
                if pipelined:
                    # emit g/u of block i+1 before down of block i
                    pend = None  # (hh, tok0, TB)
                    for bi, (tok0, TB) in enumerate(blocks):
                        if wd_spread and bi == 0:
                            load_wd(range(NWD // 2))
                        hh = gate_up_phase(tok0, TB)
                        if wd_spread and bi == 0:
                            load_wd(range(NWD // 2, NWD))
                        if pend is not None:
                            down_phase(wd_sb, *pend)
                        pend = (hh, tok0, TB)
                    down_phase(wd_sb, *pend)
                else:
                    for tok0, TB in blocks:
                        hh = gate_up_phase(tok0, TB)
                        down_phase(wd_sb, hh, tok0, TB)
    nc.compile()
    return nc


def _build_wide(
    C: int,
    reps: int = 1,
    w_bufs: int = 4,
    op_bufs: int = 2,
    **_ignored,
):
    """1024-token superblocks: paired PSUM halves per logical tile so each
    stationary weight load feeds two consecutive matmuls; weights are
    re-streamed once per 1024 tokens (half the HBM traffic of the 512-block
    schedule); wd resident. Gate/up -> down has no PE boundary stall because
    down consumes h in f-order (oldest first)."""
    assert C % 1024 == 0
    SB = C // 1024
    dt_w = mybir.dt.bfloat16
    f32 = mybir.dt.float32
    nc = bacc.Bacc("TRN2", target_bir_lowering=False, debug=False, num_devices=N_CORES)
    xt = nc.dram_tensor("xt", [D, C], dt_w, kind="ExternalInput")
    wg = nc.dram_tensor("wg", [F, D], dt_w, kind="ExternalInput")
    wu = nc.dram_tensor("wu", [F, D], dt_w, kind="ExternalInput")
    wd = nc.dram_tensor("wd", [F, D], dt_w, kind="ExternalInput")
    yt = nc.dram_tensor("yt", [D, C], f32, kind="ExternalOutput")

    with tile.TileContext(nc) as tc:
        with ExitStack() as ctx:
            wdp = ctx.enter_context(tc.tile_pool(name="wdp", bufs=1))
            wgp = ctx.enter_context(tc.tile_pool(name="wgp", bufs=w_bufs))
            wup = ctx.enter_context(tc.tile_pool(name="wup", bufs=w_bufs))
            xp = ctx.enter_context(tc.tile_pool(name="xp", bufs=1))
            hp = ctx.enter_context(tc.tile_pool(name="hp", bufs=1))
            sp = ctx.enter_context(tc.tile_pool(name="sp", bufs=4))
            yp = ctx.enter_context(tc.tile_pool(name="yp", bufs=3))
            gp = ctx.enter_context(tc.tile_pool(name="gp", bufs=1, space="PSUM"))
            up = ctx.enter_context(tc.tile_pool(name="up", bufs=1, space="PSUM"))
            op = ctx.enter_context(tc.tile_pool(name="op", bufs=op_bufs, space="PSUM"))

            xt_r = xt.rearrange("(dc p) t -> p dc t", p=128)
            wd_r = wd.rearrange("(fc p) d -> p fc d", p=128)
            TB = 1024

            def gu_phase(tok0, after_first_chunk=None):
                x_sb = xp.tile([128, DC * TB], dt_w, tag="x")
                io_eng.dma_start(
                    x_sb[:].rearrange("p (dc t) -> p dc t", dc=DC),
                    xt_r[:, :, tok0 : tok0 + TB],
                )
                hh = []
                for f in range(FC):
                    wg_t = wgp.tile([128, D], dt_w, tag="wgc")
                    nc.sync.dma_start(wg_t[:], wg[f * 128 : (f + 1) * 128, :])
                    wu_t = wup.tile([128, D], dt_w, tag="wuc")
                    nc.sync.dma_start(wu_t[:], wu[f * 128 : (f + 1) * 128, :])
                    ps = {
                        "ga": gp.tile([128, 512], f32, tag="ga", name="ga"),
                        "gb": gp.tile([128, 512], f32, tag="gb", name="gb"),
                        "ua": up.tile([128, 512], f32, tag="ua", name="ua"),
                        "ub": up.tile([128, 512], f32, tag="ub", name="ub"),
                    }
                    for w_t, pa, pb in ((wg_t, "ga", "gb"), (wu_t, "ua", "ub")):
                        for d in range(DC):
                            w_ap = w_t[:, d * 128 : (d + 1) * 128]
                            nc.tensor.matmul(
                                ps[pa][:],
                                w_ap,
                                x_sb[:, d * TB : d * TB + 512],
                                start=(d == 0),
                                stop=(d == DC - 1),
                            )
                            nc.tensor.matmul(
                                ps[pb][:],
                                w_ap,
                                x_sb[:, d * TB + 512 : (d + 1) * TB],
                                start=(d == 0),
                                stop=(d == DC - 1),
                            )
                    h = hp.tile([128, TB], dt_w, tag=f"hh{f}", name=f"hh{f}")
                    for half, (pg, pu) in enumerate((("ga", "ua"), ("gb", "ub"))):
                        sg = sp.tile([128, 512], dt_w, tag="sg", name="sg")
                        nc.scalar.activation(
                            sg[:], ps[pg][:], mybir.ActivationFunctionType.Silu
                        )
                        nc.vector.tensor_mul(
                            h[:, half * 512 : (half + 1) * 512], sg[:], ps[pu][:]
                        )
                    hh.append(h)
                    if f == 0 and after_first_chunk is not None:
                        after_first_chunk()
                return hh

            def down_phase(wd_sb, hh, tok0):
                for d in range(DC):
                    ya = op.tile([128, 512], f32, tag="ya", name="ya")
                    yb = op.tile([128, 512], f32, tag="yb", name="yb")
                    for f in range(FC):
                        w_ap = wd_sb[f // 8][
                            :, (f % 8) : (f % 8) + 1, d * 128 : (d + 1) * 128
                        ]
                        nc.tensor.matmul(
                            ya[:], w_ap, hh[f][:, :512],
                            start=(f == 0), stop=(f == FC - 1),
                        )
                        nc.tensor.matmul(
                            yb[:], w_ap, hh[f][:, 512:],
                            start=(f == 0), stop=(f == FC - 1),
                        )
                    for half, yps in enumerate((ya, yb)):
                        y_sb = yp.tile([128, 512], f32, tag="ysb", name="ysb")
                        nc.vector.tensor_copy(y_sb[:], yps[:])
                        io_eng.dma_start(
                            yt[
                                d * 128 : (d + 1) * 128,
                                tok0 + half * 512 : tok0 + (half + 1) * 512,
                            ],
                            y_sb[:],
                        )

            io_eng = nc.gpsimd

            for _rep in range(reps):
                wd_sb = []

                def load_wd(ks):
                    for k in ks:
                        t = wdp.tile([128, 8, D], dt_w, tag=f"wd{k}", name=f"wd_t{k}")
                        nc.sync.dma_start(t[:], wd_r[:, 8 * k : 8 * k + 8, :])
                        wd_sb.append(t)

                load_wd([0, 1])
                for sb in range(SB):
                    hh = gu_phase(
                        sb * 1024,
                        after_first_chunk=(lambda: load_wd([2, 3])) if sb == 0 else None,
                    )
                    down_phase(wd_sb, hh, sb * 1024)
    nc.compile()
    return nc


def _tile_gate_weights(w_t: np.ndarray) -> np.ndarray:
    """[D, F] -> [F, D] tiled so row f*128+p, col d*128+j = w_t[d*128+p, f*128+j]."""
    return (
        w_t.reshape(DC, 128, FC, 128).transpose(2, 1, 0, 3).reshape(F, D)
    )


def _ffn_host(x_rows, Wg, Wu, Wd):
    """Exact f32 SwiGLU FFN on host for overflow tokens."""
    g = x_rows @ Wg.T
    u = x_rows @ Wu.T
    h = (g / (1.0 + np.exp(-g))) * u
    return h @ Wd.T


# Device capacity policy: prefer a clean multiple of 512 token blocks and
# compute the (tiny) overflow beyond it on host; fall back to padding the
# device capacity up when overflow would be non-negligible.
OVERFLOW_FRAC_MAX = 0.02


def kernel(x, W_router, W_gate, W_up, W_down):
    bf16 = ml_dtypes.bfloat16
    x = np.asarray(x, np.float32)
    W_router = np.asarray(W_router, np.float32)
    W_gate = np.asarray(W_gate, np.float32)
    W_up = np.asarray(W_up, np.float32)
    W_down = np.asarray(W_down, np.float32)

    T = B * S
    x_flat = x.reshape(T, D)
    idx, gates = _route(x_flat, W_router)

    # token lists per expert
    tok_lists = []
    gate_lists = []
    for e in range(E):
        sel = np.nonzero(idx == e)  # (token_rows, k_pos)
        rows = sel[0]
        tok_lists.append(rows)
        gate_lists.append(gates[sel[0], sel[1]])

    max_load = max(len(r) for r in tok_lists)
    C_pad = max(128, int(np.ceil(max_load / 128)) * 128)
    C_512 = max(512, (max_load // 512) * 512)
    overflow = sum(max(0, len(r) - C_512) for r in tok_lists)
    if overflow <= OVERFLOW_FRAC_MAX * T * TOPK:
        C = C_512
    else:
        C = C_pad

    if C not in _cache:
        _cache[C] = _build(C, **BEST_KW)
    nc = _cache[C]

    in_maps = []
    for e in range(E):
        rows = tok_lists[e][:C]
        xg = np.zeros((C, D), np.float32)
        xg[: len(rows)] = x_flat[rows]
        in_maps.append(
            {
                "xt": np.ascontiguousarray(xg.T).astype(bf16),
                "wg": np.ascontiguousarray(
                    _tile_gate_weights(W_gate[e].T.astype(np.float32))
                ).astype(bf16),
                "wu": np.ascontiguousarray(
                    _tile_gate_weights(W_up[e].T.astype(np.float32))
                ).astype(bf16),
                "wd": np.ascontiguousarray(W_down[e].T).astype(bf16),
            }
        )

    try:
        res = run_bass_kernel_spmd(nc, in_maps, core_ids=list(range(N_CORES)))
    except Exception:
        # transient device failures (e.g. NRT exec-unit unrecoverable) have
        # been observed on this tunnel; one retry usually succeeds
        res = run_bass_kernel_spmd(nc, in_maps, core_ids=list(range(N_CORES)))

    out = np.zeros((T, D), np.float32)
    for e in range(E):
        rows = tok_lists[e]
        n_dev = min(len(rows), C)
        y_e = res.results[e]["yt"].T[:n_dev]  # [n_dev, D]
        out[rows[:n_dev]] += gate_lists[e][:n_dev, None] * y_e
        if len(rows) > C:  # overflow tokens -> exact host FFN
            orows = rows[C:]
            y_o = _ffn_host(x_flat[orows], W_gate[e], W_up[e], W_down[e])
            out[orows] += gate_lists[e][C:, None] * y_o
    return out.reshape(B, S, D)

